# revision 22
# baseline (speedup 1.0000x reference)
"""CLIF spiking-neuron recurrence kernel for 8 Trainium2 NeuronCores.

Reference semantics (per element, T=64 sequential steps, gamma=0.5):
    u     = 0.5*u + x_t
    spike = (u >= 1.0)
    m     = s_prev * sigmoid(0.5*u) + spike
    s     = sigmoid(m)                       # carried (in-place sigmoid_)
    u     = u - spike*(1.0 + s)
Output: spikes [T, B, D] float32.

Strategy:
- Pure data parallel over the B*D = 524288 elements: 65536 per core as
  [128 partitions x 512 free], split into G independent pipeline groups
  along the free dim. Each group's step is a serial dependency loop
  (sigmoid -> CLIF_M -> sigmoid -> CLIF_Y -> matmul); with the input
  matmul hoisted off that loop, the kernel is latency-bound at
  T * loop-latency, and groups overlap on the engines.
- The membrane potential lives in PSUM as V_t = 2^t * u_t (power-of-2
  scaling is exact in fp32; 2^63*|u|max is far below fp32 range). The
  leak folds into per-step constants; the input add V += I @ (2^t x_t)
  runs on the TensorEngine right after step t-1's sigmoid read, off the
  critical loop; the reset matmul V += I @ y closes the loop.
- Two custom DVE ops (registered into the per-NEFF uop table, verified
  bit-exact on HW) fuse all elementwise work into 2 Vector ops:
    CLIF_M: M = s_prev*sg + (sg >= c)            (q-mult + spike + add)
    CLIF_Y: y = (sg >= c) * (s*(-2^t) + (-2^t))  [= -2^t*spike*(1+s)]
  where sg = sigmoid(2^-(t+1) * V) and c = sigmoidLUT(0.5) is computed
  on-device once; the ACT LUT is strictly monotone around z=0.5
  (verified on HW), so (sg >= c) <=> (u >= 1) exactly.
- One wide [128,512] input DMA and one wide output DMA per step.
- The kernel streams out sg (fp32); the host applies spike = (sg >= c),
  bit-identical to the on-device compares.
"""

import sys
import types

import numpy as np
import ml_dtypes

# If BASS_TRACE is set but the image's antenv lacks axon_hooks,
# run_bass_kernel_spmd would crash importing it; install a null-hook
# module so tracing degrades gracefully instead.
try:
    import antenv.axon_hooks  # noqa: F401
except Exception:
    try:
        import antenv
        _hooks = types.ModuleType("antenv.axon_hooks")
        _hook_cell = [None]
        _hooks.set_axon_ntff_profile_hook = (
            lambda h: _hook_cell.__setitem__(0, h))
        _hooks.get_axon_ntff_profile_hook = lambda: _hook_cell[0]
        sys.modules["antenv.axon_hooks"] = _hooks
        antenv.axon_hooks = _hooks
    except Exception:
        pass

import concourse.bass as bass
import concourse.bacc as bacc
import concourse.mybir as mybir
import concourse.tile as tile
import concourse.dve_ops as dve_ops
from concourse.dve_spec import Spec, Src0, Src1, C0, C1, lower, _has_src1
from concourse.dve_uop import DveOpSpec
from concourse.bass_utils import run_bass_kernel_spmd

F32 = mybir.dt.float32
AF = mybir.ActivationFunctionType
ALU = mybir.AluOpType

T = 64
B = 128
D = 4096
N_CORES = 8
P = 128
NPC = B * D // N_CORES          # 65536 elements per core
FDT = NPC // P                  # 512 free columns per core

# group column ranges (start, width) along the 512-wide free dim
GROUPS = [(0, 172), (172, 172), (344, 168)]

_NC_CACHE = None
LAST_RESULTS = None


def _register_dve_op(name, spec):
    for op in dve_ops.OPS:
        if op.name == name:
            return op
    shas = {}
    for ver in ("v3", "v4"):
        u = lower(spec, ver=ver)
        shas[ver] = DveOpSpec(name=name, opcode=1, uops=u,
                              rd1_en=_has_src1(spec)).sha(ver)
    op = dve_ops.DveOp(name, spec, subdim=False, uops_sha=shas)
    dve_ops.OPS.append(op)
    dve_ops._SUB_OPCODE_FOR_NAME[name] = (
        dve_ops._CUSTOM_DVE_ROW_BASE + len(dve_ops.OPS) - 1)
    dve_ops.CUSTOM_DVE_SPECS[name] = spec
    return op


# M = s_prev*sg + (sg >= c)          in0=s_prev, in1=sg, s0=c
CLIF_M = _register_dve_op("CLIF_M_ANT", Spec(
    body=Src0 * Src1 + (Src1 >= C0),
    reference=lambda in0, in1, s0, s1, imm2:
        in0 * in1 + (in1 >= s0).astype(np.float32),
))
# y = (sg >= c) * (s*zneg + zneg)    in0=s, in1=sg, s0=c, s1=zneg=-2^t
CLIF_Y = _register_dve_op("CLIF_Y_ANT", Spec(
    body=(Src1 >= C0) * (Src0 * C1 + C1),
    reference=lambda in0, in1, s0, s1, imm2:
        (in1 >= s0).astype(np.float32) * (in0 * s1 + s1),
))


def _build():
    nc = bacc.Bacc(None, target_bir_lowering=False, debug=False,
                   num_devices=N_CORES)

    xs = nc.declare_dram_parameter("xs", [T, P, FDT], F32, isOutput=False)
    wt = nc.declare_dram_parameter("wt", [P, P], F32, isOutput=False)  # identity
    out = nc.declare_dram_parameter("out", [T, P, FDT], F32, isOutput=True)
    cout = nc.declare_dram_parameter("cout", [P, 1], F32, isOutput=True)

    G = len(GROUPS)
    with tile.TileContext(nc) as tc:
        with (
            tc.tile_pool(name="wpool", bufs=1) as wpool,
            tc.tile_pool(name="cpool", bufs=1) as cpool,
            tc.tile_pool(name="xpool", bufs=8) as xpool,
            tc.tile_pool(name="sgpool", bufs=6) as sgpool,
            tc.tile_pool(name="spool", bufs=4) as spool,
            tc.tile_pool(name="mpool", bufs=4) as mpool,
            tc.tile_pool(name="ypool", bufs=4) as ypool,
            tc.tile_pool(name="vpool", bufs=1, space="PSUM") as vpool,
        ):
            # --- one-time setup -------------------------------------------
            eye = wpool.tile([P, P], F32, tag="eye")
            nc.sync.dma_start(eye[:], wt[:])

            halft = cpool.tile([P, 1], F32, tag="half")
            nc.gpsimd.memset(halft[:], 0.5)
            ct = cpool.tile([P, 1], F32, tag="c")
            # c = sigmoid_LUT(0.5), same LUT as the per-step sigmoids
            nc.scalar.activation(ct[:], halft[:], AF.Sigmoid, bias=0.0, scale=1.0)
            nc.sync.dma_start(cout[:], ct[:])
            c_ap = ct[:, 0:1]

            # --- initial state --------------------------------------------
            V = []
            s_prev = []
            for g, (o, w) in enumerate(GROUPS):
                s0 = spool.tile([P, w], F32, tag=f"s{g}")
                nc.gpsimd.memset(s0[:], 0.0)
                s_prev.append(s0)
                vt = vpool.tile([P, w], F32, tag=f"V{g}")
                V.append(vt)

            # PE warm-up: dummy matmuls fill the otherwise-idle prologue
            # window so the HAM clock gate reaches 2.4 GHz before the first
            # real matmul (the first ~8 steps otherwise run at 1.2 GHz)
            junk = vpool.tile([P, 128], F32, tag="junk")
            for _ in range(10):
                nc.tensor.matmul(junk[:], eye[:], eye[:], start=True, stop=True)

            x0 = xpool.tile([P, FDT], F32, tag="x")
            nc.sync.dma_start(x0[:], xs[0])
            for g, (o, w) in enumerate(GROUPS):
                nc.tensor.matmul(V[g][:], eye[:], x0[:, o:o + w],
                                 start=True, stop=False, skip_group_check=True)

            # --- the recurrence -------------------------------------------
            for t in range(T):
                sc_sg = float(2.0 ** (-t - 1))
                zneg = float(-(2.0 ** t))

                # one wide input prefetch per step (all groups)
                if t < T - 1:
                    xnext = xpool.tile([P, FDT], F32, tag="x")
                    nc.sync.dma_start(xnext[:], xs[t + 1])

                # one wide output tile per step; ACT fills per-group slices.
                # Ops are emitted stage-grouped across groups so no engine's
                # FIFO head-of-line blocks an independent group's work.
                sgw = sgpool.tile([P, FDT], F32, tag="sg")
                for g, (o, w) in enumerate(GROUPS):
                    nc.scalar.activation(sgw[:, o:o + w], V[g][:], AF.Sigmoid,
                                         bias=0.0, scale=sc_sg)

                if t < T - 1:
                    # input add for the NEXT step: off the critical loop,
                    # legal as soon as this step's sigmoid has read V
                    for g, (o, w) in enumerate(GROUPS):
                        nc.tensor.matmul(V[g][:], eye[:], xnext[:, o:o + w],
                                         start=False, stop=False,
                                         skip_group_check=True)

                    # M = s_prev*sg + spike
                    msbs = []
                    for g, (o, w) in enumerate(GROUPS):
                        msb = mpool.tile([P, w], F32, tag=f"m{g}")
                        nc.vector._custom_dve(CLIF_M, out=msb[:],
                                              in0=s_prev[g][:],
                                              in1=sgw[:, o:o + w], s0=c_ap)
                        msbs.append(msb)

                    # s = sigmoid(M)
                    for g, (o, w) in enumerate(GROUPS):
                        s_new = spool.tile([P, w], F32, tag=f"s{g}")
                        nc.scalar.activation(s_new[:], msbs[g][:], AF.Sigmoid,
                                             bias=0.0, scale=1.0)
                        s_prev[g] = s_new

                    # y = -2^t * spike * (1+s) ; V += I @ y closes the loop
                    for g, (o, w) in enumerate(GROUPS):
                        y = ypool.tile([P, w], F32, tag=f"y{g}")
                        nc.vector._custom_dve(CLIF_Y, out=y[:],
                                              in0=s_prev[g][:],
                                              in1=sgw[:, o:o + w],
                                              s0=c_ap, s1=zneg)
                        nc.tensor.matmul(V[g][:], eye[:], y[:],
                                         start=False, stop=(t + 1 == T - 1),
                                         skip_group_check=True)

                nc.sync.dma_start(out[t], sgw[:])

    nc.compile()
    return nc


def _get_nc():
    global _NC_CACHE
    if _NC_CACHE is None:
        _NC_CACHE = _build()
    return _NC_CACHE


def kernel(x_seq: np.ndarray) -> np.ndarray:
    global LAST_RESULTS
    x = np.ascontiguousarray(x_seq, dtype=np.float32)
    assert x.shape == (T, B, D), x.shape

    # 2^t prescale (exact in fp32) and per-core shard [T, P, FDT]
    scale = (2.0 ** np.arange(T, dtype=np.float64)).astype(np.float32)
    xsc = x.reshape(T, -1) * scale[:, None]
    xsc = xsc.reshape(T, N_CORES, P, FDT)

    eye_host = np.eye(P, dtype=np.float32)

    nc = _get_nc()
    in_maps = [
        {"xs": np.ascontiguousarray(xsc[:, c]), "wt": eye_host}
        for c in range(N_CORES)
    ]
    LAST_RESULTS = run_bass_kernel_spmd(nc, in_maps, list(range(N_CORES)))

    full = np.empty((T, N_CORES, P, FDT), dtype=np.float32)
    for c in range(N_CORES):
        res = LAST_RESULTS.results[c]
        c_val = np.asarray(res["cout"], dtype=np.float32)[0, 0]
        sg = np.asarray(res["out"], dtype=np.float32)
        full[:, c] = (sg >= c_val).astype(np.float32)
    return full.reshape(T, B, D)
